# revision 2
# baseline (speedup 1.0000x reference)
import sys, time
sys.path.insert(0, "/opt/trn_rl_repo")
import numpy as np
from concourse import bass, bacc, mybir, tile
from concourse.bass_utils import run_bass_kernel_spmd

# Problem constants (nn_Memory_88656714925588)
B, CK, CV = 1, 64, 256
H, W, T = 64, 64, 8
NE = H * W * T            # 32768
Q = H * W                 # 4096
NC = 8                    # cores
NE_LOC = NE // NC         # 4096 memory elements per core
Q_LOC = Q // NC           # 512 queries per core in phase 3
TOPK = 20
NG = 2                    # groups per tile (one per 2048-wide psum half)
NCAND = NG * 8            # 16 candidates per (query, core)
NSLOT = NC * NCAND        # 128 candidate slots per query after all-to-all
NQT = Q // 128            # 32 query tiles in phase 1
NQT3 = Q_LOC // 128       # 4 query tiles per core in phase 3
F32 = mybir.dt.float32
F16 = mybir.dt.float16
I16 = mybir.dt.int16
U32 = mybir.dt.uint32
NEG = -1e30

_prog_cache = {}


def _build_program():
    if "p" in _prog_cache:
        return _prog_cache["p"]
    nc = bacc.Bacc()
    # fp16 split-precision stacks (host-prepared)
    qA = nc.dram_tensor("qA", [128, Q], F16, kind="ExternalInput")      # [q_hi; q_lo]
    qB = nc.dram_tensor("qB", [66, Q], F16, kind="ExternalInput")       # [q_hi; 1; 1]
    mA = nc.dram_tensor("mA", [128, NE_LOC], F16, kind="ExternalInput")  # [m_hi; m_hi]
    mB = nc.dram_tensor("mB", [66, NE_LOC], F16, kind="ExternalInput")  # [m_lo; b_hi; b_lo]
    gnc = nc.dram_tensor("gnc", [128, NCAND], F32, kind="ExternalInput")  # idx base/slot
    rk16 = nc.dram_tensor("rk16", [128, TOPK], I16, kind="ExternalInput")  # ranks 1..20
    vT = nc.dram_tensor("vT", [NE, 2 * CV], F32, kind="ExternalInput")
    out = nc.dram_tensor("out", [Q_LOC, 2 * CV], F32, kind="ExternalOutput")

    with tile.TileContext(nc) as tc:
        with tc.tile_pool(name="sbuf", bufs=2) as pool, \
             tc.tile_pool(name="p3", bufs=2) as p3, \
             tc.tile_pool(name="cst", bufs=1) as cst, \
             tc.tile_pool(name="gkp", bufs=2) as gkp, \
             tc.tile_pool(name="psum", bufs=2, space="PSUM") as psum, \
             tc.tile_pool(name="dram", bufs=2, space="DRAM") as dram:

            qAt = cst.tile([128, Q], F16)
            qBt = cst.tile([66, Q], F16)
            mAt = cst.tile([128, NE_LOC], F16)
            mBt = cst.tile([66, NE_LOC], F16)
            # chunked input loads so the first matmul can start early
            for ci in range(4):
                sl = slice(ci * 1024, (ci + 1) * 1024)
                nc.sync.dma_start(out=mAt[:, sl], in_=mA[:, sl])
                nc.sync.dma_start(out=mBt[:, sl], in_=mB[:, sl])
            for ci in range(4):
                sl = slice(ci * 1024, (ci + 1) * 1024)
                nc.scalar.dma_start(out=qAt[:, sl], in_=qA[:, sl])
                nc.scalar.dma_start(out=qBt[:, sl], in_=qB[:, sl])
            gb = cst.tile([128, NCAND], F32)
            nc.sync.dma_start(out=gb[:], in_=gnc[:])
            rkt = cst.tile([128, TOPK], I16)
            nc.sync.dma_start(out=rkt[:], in_=rk16[:])

            candL = dram.tile([Q, 2 * NCAND], F32)
            candX = dram.tile([Q, 2 * NCAND], F32)
            wrap_d = dram.tile([NQT3, TOPK * 128], I16)   # wrapped gather idx per ph3 tile

            # ---------------- Phase 3: merge + scatter-extract + readout ----
            def phase3(tt):
                cG = p3.tile([128, NC * 2 * NCAND], F32, tag="cG")
                nc.sync.dma_start(
                    out=cG[:],
                    in_=candX[tt * NC * 128:(tt + 1) * NC * 128, :]
                    .rearrange("(g p) c -> p g c", p=128))
                sv = cG[:].rearrange("p (u two) -> p u two", two=2)[:, :, 0]
                # exact top-20 of the 128 slots: 3 rounds of max8 + replace
                work = p3.tile([128, NSLOT], F32, tag="work")
                nc.vector.tensor_copy(work[:], sv)
                gvals = p3.tile([128, 24], F32, tag="gvals")
                for r in range(3):
                    m8 = gvals[:, r * 8:(r + 1) * 8]
                    nc.vector.max(out=m8, in_=work[:])
                    if r < 2:
                        nc.vector.match_replace(
                            out=work[:], in_to_replace=m8, in_values=work[:],
                            imm_value=NEG)
                gpos = p3.tile([128, 24], U32, tag="gpos")
                for r in range(3):
                    nc.vector.max_index(
                        out=gpos[:, r * 8:(r + 1) * 8],
                        in_max=gvals[:, r * 8:(r + 1) * 8], in_values=sv)
                # softmax over the top-20 values
                negm = p3.tile([128, 1], F32, tag="negm")
                nc.vector.tensor_scalar(
                    negm[:], gvals[:, 0:1], -1.0, None, op0=mybir.AluOpType.mult)
                wexp = p3.tile([128, TOPK], F32, tag="wexp")
                ssum = p3.tile([128, 1], F32, tag="ssum")
                nc.scalar.activation(
                    out=wexp[:], in_=gvals[:, :TOPK],
                    func=mybir.ActivationFunctionType.Exp,
                    bias=negm[:], scale=1.0, accum_out=ssum[:])
                rs = p3.tile([128, 1], F32, tag="rs")
                nc.vector.reciprocal(rs[:], ssum[:])
                wgt = p3.tile([128, TOPK], F32, tag="wgt")
                nc.vector.tensor_scalar(
                    wgt[:], wexp[:], rs[:], None, op0=mybir.AluOpType.mult)
                # ---- extract winner element-indices via double local_scatter
                gp16 = p3.tile([128, TOPK], I16, tag="gp16")
                nc.vector.tensor_copy(gp16[:], gpos[:, :TOPK])
                rslot = p3.tile([128, NSLOT], I16, tag="rslot")
                nc.gpsimd.local_scatter(
                    out_ap=rslot[:], data_ap=rkt[:], idxs_ap=gp16[:],
                    channels=128, num_elems=NSLOT, num_idxs=TOPK)
                rmap = p3.tile([128, NSLOT], I16, tag="rmap")
                nc.vector.tensor_scalar(
                    rmap[:], rslot[:], -1, None, op0=mybir.AluOpType.add)
                ci16 = p3.tile([128, NSLOT], I16, tag="ci16")
                nc.vector.tensor_copy(
                    ci16[:], cG[:].rearrange("p (u two) -> p u two", two=2)[:, :, 1])
                widx = p3.tile([128, TOPK], I16, tag="widx")
                nc.gpsimd.local_scatter(
                    out_ap=widx[:], data_ap=ci16[:], idxs_ap=rmap[:],
                    channels=128, num_elems=TOPK, num_idxs=NSLOT)
                # ---- wrapped-index DRAM bounce for dma_gather
                # DRAM layout: flat[k*128 + p] = widx[p, k]  (k-major)
                nc.sync.dma_start(
                    out=wrap_d[tt:tt + 1, :].rearrange("one (k p) -> (one p) k",
                                                       p=128),
                    in_=widx[:])
                # wrapped view: wri[pp, k*8+cb] = flat[k*128 + cb*16 + pp]
                wri = p3.tile([128, TOPK * 8], I16, tag="wri")
                nc.scalar.dma_start(
                    out=wri[0:16, :],
                    in_=wrap_d[tt:tt + 1, :].rearrange(
                        "one (k cb pp) -> (one pp) (k cb)", pp=16, cb=8, k=TOPK))
                nc.sync.dma_start(out=wri[16:32, :], in_=wri[0:16, :])
                nc.scalar.dma_start(out=wri[32:64, :], in_=wri[0:32, :])
                nc.sync.dma_start(out=wri[64:128, :], in_=wri[0:64, :])
                # ---- V-row gather in two halves, pipelined with readout
                last = tt == NQT3 - 1
                KH = TOPK // 2
                acc = p3.tile([128, 2 * CV], F32, tag="acc")
                nc.gpsimd.memset(acc[:], 0.0)
                if last:
                    acc2 = p3.tile([128, 2 * CV], F32, tag="acc2")
                    nc.vector.memset(acc2[:], 0.0)
                for hh in range(2):
                    gk = gkp.tile([128, KH * 2 * CV], F32, tag=f"gk{hh}")
                    nc.gpsimd.dma_gather(
                        out_ap=gk[:].rearrange("p (k c) -> p k c", k=KH),
                        in_ap=vT[:],
                        idxs_ap=wri[:, hh * KH * 8:(hh + 1) * KH * 8],
                        num_idxs=KH * 128, num_idxs_reg=KH * 128,
                        elem_size=2 * CV)
                    gk3 = gk[:].rearrange("p (k c) -> p k c", k=KH)
                    for k in range(KH):
                        kk = hh * KH + k
                        # on the last chunk, the DVE is idle: split the
                        # accumulation across Pool and DVE to shorten the tail
                        if last and (k % 2 == 1):
                            nc.vector.scalar_tensor_tensor(
                                out=acc2[:], in0=gk3[:, k, :],
                                scalar=wgt[:, kk:kk + 1], in1=acc2[:],
                                op0=mybir.AluOpType.mult,
                                op1=mybir.AluOpType.add)
                        else:
                            nc.gpsimd.scalar_tensor_tensor(
                                out=acc[:], in0=gk3[:, k, :],
                                scalar=wgt[:, kk:kk + 1], in1=acc[:],
                                op0=mybir.AluOpType.mult,
                                op1=mybir.AluOpType.add)
                if last:
                    nc.vector.tensor_tensor(out=acc[:], in0=acc[:], in1=acc2[:],
                                            op=mybir.AluOpType.add)
                nc.scalar.dma_start(
                    out=out[tt * 128:(tt + 1) * 128, :], in_=acc[:])

            # ---------------- Phase 1: affinity + per-half exact top-8 ------
            # tile order: chunk-major (j, d), t = d*NQT3 + j
            # phase3(j) is deferred by 2 tiles so its cG-load wait never
            # head-of-line-blocks the DVE queue.
            _order = [d * NQT3 + j for j in range(NQT3) for d in range(NC)]
            pending = []
            for ti, t in enumerate(_order):
                crow = pool.tile([128, 2 * NCAND], F32, tag="crow", bufs=4)
                for h in range(2):
                    ph = psum.tile([128, 2048], F32, tag="ph")
                    for c in range(4):
                        sl = slice(h * 2048 + c * 512, h * 2048 + (c + 1) * 512)
                        po = ph[:, c * 512:(c + 1) * 512]
                        nc.tensor.matmul(
                            out=po, lhsT=qAt[:, t * 128:(t + 1) * 128],
                            rhs=mAt[:, sl], start=True, stop=False)
                        nc.tensor.matmul(
                            out=po, lhsT=qBt[:, t * 128:(t + 1) * 128],
                            rhs=mBt[:, sl], start=False, stop=True)
                    # stage PSUM -> SBUF on the scalar engine, then scan from
                    # SBUF: frees PSUM fast (PE runs ahead) and decouples the
                    # DVE from the PE/PSUM handshake
                    st = pool.tile([128, 2048], F32, tag=f"st{h}", bufs=2)
                    nc.scalar.copy(out=st[:], in_=ph[:])
                    vsl = crow[:].rearrange("p (u two) -> p u two", two=2)[:, h * 8:(h + 1) * 8, 0]
                    nc.vector.max(out=vsl, in_=st[:])
                    iu = pool.tile([128, 8], U32, tag="iu")
                    nc.vector.max_index(out=iu[:], in_max=vsl, in_values=st[:])
                    iuf = pool.tile([128, 8], F32, tag="iuf")
                    nc.vector.tensor_copy(iuf[:], iu[:])
                    nc.vector.scalar_tensor_tensor(
                        out=crow[:].rearrange("p (u two) -> p u two", two=2)[:, h * 8:(h + 1) * 8, 1],
                        in0=iuf[:], scalar=1.0, in1=gb[:, h * 8:(h + 1) * 8],
                        op0=mybir.AluOpType.mult, op1=mybir.AluOpType.add)
                j, d = t % NQT3, t // NQT3
                row = (j * NC + d) * 128
                nc.sync.dma_start(out=candL[row:row + 128, :], in_=crow[:])
                if ti % NC == NC - 1:
                    nc.gpsimd.collective_compute(
                        "AllToAll", mybir.AluOpType.bypass,
                        replica_groups=[list(range(NC))],
                        ins=[candL[j * NC * 128:(j + 1) * NC * 128, :].opt()],
                        outs=[candX[j * NC * 128:(j + 1) * NC * 128, :].opt()])
                    pending.append(j)
                if pending and (ti >= len(_order) - 1 or ti % NC == 1):
                    phase3(pending.pop(0))
            while pending:
                phase3(pending.pop(0))
    nc.finalize()
    _prog_cache["p"] = nc
    return nc


def _host_prep(qk, mem_k, mem_v1, mem_v2):
    q2 = qk.reshape(CK, Q).astype(np.float32)
    qs = 0.25 * q2
    q_hi = qs.astype(np.float16)
    q_lo = (qs - q_hi.astype(np.float32)).astype(np.float16)
    ones = np.ones((2, Q), np.float16)
    qA = np.concatenate([q_hi, q_lo], axis=0)                      # [128, Q] f16
    qB = np.concatenate([q_hi, ones], axis=0)                      # [66, Q] f16

    mk = mem_k[0].astype(np.float32)                               # [64, NE]
    a = np.sum(mk * mk, axis=0, dtype=np.float32)
    bias = -0.125 * a                                              # [NE]
    m_hi = mk.astype(np.float16)
    m_lo = (mk - m_hi.astype(np.float32)).astype(np.float16)
    b_hi = bias.astype(np.float16)
    b_lo = (bias - b_hi.astype(np.float32)).astype(np.float16)

    vT = np.concatenate([mem_v1[0].T, mem_v2[0].T], axis=1).astype(np.float32)
    rk16 = np.broadcast_to(np.arange(1, TOPK + 1, dtype=np.int16), (128, TOPK)).copy()

    in_maps = []
    for c in range(NC):
        sl = slice(c * NE_LOC, (c + 1) * NE_LOC)
        mA = np.concatenate([m_hi[:, sl], m_hi[:, sl]], axis=0)    # [128, NE_LOC]
        mB = np.concatenate(
            [m_lo[:, sl], b_hi[None, sl], b_lo[None, sl]], axis=0)  # [66, NE_LOC]
        gslot = np.empty(NCAND, np.float32)
        for si in range(NCAND):
            gslot[si] = c * NE_LOC + (si // 8) * 2048
        in_maps.append({
            "qA": qA, "qB": qB,
            "mA": np.ascontiguousarray(mA), "mB": np.ascontiguousarray(mB),
            "gnc": np.broadcast_to(gslot, (128, NCAND)).astype(np.float32).copy(),
            "rk16": rk16, "vT": vT,
        })
    return in_maps


def kernel(qk, mem_k, mem_v1, mem_v2, top_k):
    assert int(top_k) == TOPK
    qk = np.asarray(qk, dtype=np.float32)
    mem_k = np.asarray(mem_k, dtype=np.float32)
    mem_v1 = np.asarray(mem_v1, dtype=np.float32)
    mem_v2 = np.asarray(mem_v2, dtype=np.float32)

    in_maps = _host_prep(qk, mem_k, mem_v1, mem_v2)
    nc = _build_program()
    res = None
    for attempt in range(3):
        try:
            res = run_bass_kernel_spmd(nc, in_maps, core_ids=list(range(NC)))
            break
        except Exception:
            if attempt == 2:
                raise
            time.sleep(2.0)
    full = np.concatenate([res.results[c]["out"] for c in range(NC)], axis=0)
    return np.ascontiguousarray(full.T).reshape(1, 2 * CV, H, W)
